# revision 6
# baseline (speedup 1.0000x reference)
"""CrossAttention Trainium2 kernel (8 NeuronCores, SPMD).

Sharding: core c handles batch b = c//4 and head-group hg = c%4 (4 of 16 heads,
256 of 1024 output channels). Each core computes a partial out-projection
y_part[b] = softmax(q_hg k_hg^T * scale) v_hg @ wo[:, hg].T ; the host sums the
4 head-group partials per batch and adds bo.

Device layout (per core):
  - activations are passed pre-transposed (qT/xT: [C, N]) so projection
    matmuls contract C on the partition axis with no on-chip transposes.
  - q^T, k^T produced in [o, i] layout (o = head-group channel on partitions),
    v in natural [j, o] layout augmented with a 64-wide ones block per head so
    the A@V matmul also produces the softmax row-sums (partitions 64-127).
  - S^T = k q^T per (head, j-tile): row-packed matmul pairs (d=64 contraction
    at partition bases 0/64 run concurrently on the PE sub-arrays).
  - exp on ScalarE with the 1/sqrt(C) scale folded in, PSUM -> SBUF f32r.
  - A@V accumulates O^T (+ sums) in PSUM over the 16 j-tiles.
  - normalize rows by 1/sum, out-proj against wo^T, stream y to DRAM.
All matmuls run in float32r (full PE rate for N>=256, ~1e-4 relative rounding).
"""

import sys

sys.path.insert(0, "/opt/trn_rl_repo")

import numpy as np

import concourse.bass as bass
import concourse.mybir as mybir
import concourse.tile as tile
from concourse.bass_utils import run_bass_kernel_spmd

F32 = mybir.dt.float32
F32R = mybir.dt.float32r

B = 2
N = 2048
C = 1024
H = 16
D = 64
SCALE = C ** (-0.5)
HG = 4          # head-groups (shards per batch)
HG_HEADS = 4    # heads per group
O = HG_HEADS * D  # 256 channels per group
NT = N // 128   # 16 j-tiles
EXP_FN = mybir.ActivationFunctionType.Exp


def _split_waits(nc, max_waits=1):
    """The installed walrus rejects >1 semaphore wait per instruction
    ("Too many sync wait commands"). Split extra waits onto same-engine
    NoOps inserted immediately before the instruction."""
    n_nops = 0
    for f in nc.m.functions:
        for b in f.blocks:
            new_insts = []
            for inst in b.instructions:
                si = inst.sync_info
                waits = list(si.on_wait) if si is not None else []
                if len(waits) > max_waits:
                    head, tail = waits[:-max_waits], waits[-max_waits:]
                    for i in range(0, len(head), max_waits):
                        nop = mybir.InstNoOp(name=f"WS-{n_nops}", ins=[], outs=[])
                        n_nops += 1
                        nop.engine = inst.engine
                        nop.sync_info = mybir.SyncInfo(
                            on_wait=head[i : i + max_waits], on_update=[]
                        )
                        new_insts.append(nop)
                        nc.register_instruction(nop, overwrite=True)
                    inst.sync_info = mybir.SyncInfo(
                        on_wait=tail, on_update=list(si.on_update)
                    )
                new_insts.append(inst)
            b.instructions = new_insts


def _build():
    nc = bass.Bass()

    qT = nc.dram_tensor("qT", [C, N], F32R, kind="ExternalInput")
    xT = nc.dram_tensor("xT", [C, N], F32R, kind="ExternalInput")
    wqT = nc.dram_tensor("wqT", [C, O], F32R, kind="ExternalInput")
    wkT = nc.dram_tensor("wkT", [C, O], F32R, kind="ExternalInput")
    wvT = nc.dram_tensor("wvT", [C, O], F32R, kind="ExternalInput")
    woT = nc.dram_tensor("woT", [O, C], F32R, kind="ExternalInput")
    bqt = nc.dram_tensor("bqt", [128, 2], F32, kind="ExternalInput")
    bkt = nc.dram_tensor("bkt", [128, 2], F32, kind="ExternalInput")
    bv = nc.dram_tensor("bv", [O], F32, kind="ExternalInput")
    ones = nc.dram_tensor("ones", [64], F32R, kind="ExternalInput")
    y = nc.dram_tensor("y", [N, C], F32, kind="ExternalOutput")
    import os
    dbg = os.environ.get("KERNEL_DEBUG", "0") == "1"
    if dbg:
        qTp_d = nc.dram_tensor("qTp_d", [128, 2, N], F32, kind="ExternalOutput")
        kTp_d = nc.dram_tensor("kTp_d", [128, 2, N], F32, kind="ExternalOutput")
        vaug_d = nc.dram_tensor("vaug_d", [128, NT, HG_HEADS, 128], F32, kind="ExternalOutput")
        attT_d = nc.dram_tensor("attT_d", [128, 2, N], F32, kind="ExternalOutput")

    with tile.TileContext(nc) as tc:
        with (
            tc.tile_pool(name="weights", bufs=1) as wp,
            tc.tile_pool(name="resident", bufs=1) as rp,
            tc.tile_pool(name="stream", bufs=10) as sp,
            tc.tile_pool(name="apool", bufs=3) as ap,
            tc.tile_pool(name="rpool", bufs=2) as rcp,
            tc.tile_pool(name="ysb", bufs=2) as yp,
            tc.tile_pool(name="spsum", bufs=2, space="PSUM") as sps,
            tc.tile_pool(name="opsum", bufs=2, space="PSUM") as ops,
        ):
            # ---- resident weights / constants ----
            wq_sb = wp.tile([128, 8, O], F32R, tag="wq")
            wk_sb = wp.tile([128, 8, O], F32R, tag="wk")
            wv_sb = wp.tile([128, 8, O], F32R, tag="wv")
            wo_sb = wp.tile([128, 2, C], F32R, tag="wo")
            nc.sync.dma_start(out=wq_sb, in_=wqT.rearrange("(t p) o -> p t o", p=128))
            nc.sync.dma_start(out=wk_sb, in_=wkT.rearrange("(t p) o -> p t o", p=128))
            nc.sync.dma_start(out=wv_sb, in_=wvT.rearrange("(t p) o -> p t o", p=128))
            nc.sync.dma_start(out=wo_sb, in_=woT.rearrange("(t p) u -> p t u", p=128))
            bq_sb = wp.tile([128, 2], F32, tag="bq")
            bk_sb = wp.tile([128, 2], F32, tag="bk")
            nc.sync.dma_start(out=bq_sb, in_=bqt[:])
            nc.sync.dma_start(out=bk_sb, in_=bkt[:])
            bv_sb = wp.tile([128, O], F32, tag="bv")
            nc.sync.dma_start(
                out=bv_sb, in_=bass.AP(tensor=bv, offset=0, ap=[[0, 128], [1, O]])
            )

            # ---- resident activations ----
            qTp = rp.tile([128, 2, N], F32R, tag="qTp")   # q^T  [o, i]
            kTp = rp.tile([128, 2, N], F32R, tag="kTp")   # k^T  [o, j]
            v_aug = rp.tile([128, NT, HG_HEADS, 128], F32R, tag="vaug")  # [j, jt, h, v|1]
            attT = rp.tile([128, 2, N], F32R, tag="attT")  # normalized O^T [o, i]

            # ones blocks of v_aug via broadcast DMA (f32r producer for matmul)
            v_ones = v_aug.rearrange("p a b c -> p (a b) c")[:, :, 64:128]
            nc.sync.dma_start(
                out=v_ones,
                in_=bass.AP(
                    tensor=ones,
                    offset=0,
                    ap=[[0, 128], [0, NT * HG_HEADS], [1, 64]],
                ),
            )

            # ---- phase Q: q^T projection ----
            qT_t = qT.rearrange("(ct p) i -> p ct i", p=128)
            xT_t = xT.rearrange("(ct p) i -> p ct i", p=128)
            for ich in range(4):
                i0 = ich * 512
                q01 = sps.tile([128, 2, 512], F32, tag="s")
                for ct in range(8):
                    chunk = sp.tile([128, 512], F32R, tag="chunk")
                    nc.sync.dma_start(out=chunk, in_=qT_t[:, ct, i0 : i0 + 512])
                    for t in range(2):
                        nc.tensor.matmul(
                            q01[:, t, :],
                            wq_sb[:, ct, t * 128 : (t + 1) * 128],
                            chunk[:],
                            start=(ct == 0),
                            stop=(ct == 7),
                        )
                for t in range(2):
                    nc.vector.tensor_scalar(
                        out=qTp[:, t, i0 : i0 + 512],
                        in0=q01[:, t, :],
                        scalar1=bq_sb[:, t : t + 1],
                        scalar2=None,
                        op0=mybir.AluOpType.add,
                    )

            # ---- phase KV: k^T and v projections ----
            # one PSUM accumulation group per bank at a time: start=True
            # clears the whole bank, so v j-tiles are accumulated one by one
            # over chunks kept resident in SBUF.
            for ich in range(4):
                i0 = ich * 512
                k01 = sps.tile([128, 2, 512], F32, tag="s")
                chunks = []
                for ct in range(8):
                    chunk = sp.tile([128, 512], F32R, tag="chunk", name=f"xc{ich}_{ct}")
                    nc.sync.dma_start(out=chunk, in_=xT_t[:, ct, i0 : i0 + 512])
                    chunks.append(chunk)
                    for t in range(2):
                        nc.tensor.matmul(
                            k01[:, t, :],
                            wk_sb[:, ct, t * 128 : (t + 1) * 128],
                            chunk[:],
                            start=(ct == 0),
                            stop=(ct == 7),
                        )
                for t in range(2):
                    nc.vector.tensor_scalar(
                        out=kTp[:, t, i0 : i0 + 512],
                        in0=k01[:, t, :],
                        scalar1=bk_sb[:, t : t + 1],
                        scalar2=None,
                        op0=mybir.AluOpType.add,
                    )
                for jt in range(4):
                    vj = ops.tile([128, O], F32, tag="o", name=f"vj{ich}_{jt}")
                    for ct in range(8):
                        nc.tensor.matmul(
                            vj[:],
                            chunks[ct][:, jt * 128 : (jt + 1) * 128],
                            wv_sb[:, ct, :],
                            start=(ct == 0),
                            stop=(ct == 7),
                        )
                    nc.vector.tensor_tensor(
                        out=v_aug[:, ich * 4 + jt, :, 0:64],
                        in0=vj[:].rearrange("p (h d) -> p h d", h=HG_HEADS),
                        in1=bv_sb[:].rearrange("p (h d) -> p h d", h=HG_HEADS),
                        op=mybir.AluOpType.add,
                    )

            # ---- attention + out-proj, interleaved per i-block ----
            for iblk in range(2):
                ib0 = iblk * 1024
                for pr in range(2):
                    h0, h1 = 2 * pr, 2 * pr + 1
                    Op = [
                        ops.tile([128, 1024], F32, tag="o", name=f"O{iblk}_{pr}_0"),
                        ops.tile([128, 1024], F32, tag="o", name=f"O{iblk}_{pr}_1"),
                    ]
                    for j in range(16):
                        Sp = [
                            sps.tile([128, 1024], F32, tag="s", name=f"S{iblk}_{pr}_{j}_0"),
                            sps.tile([128, 1024], F32, tag="s", name=f"S{iblk}_{pr}_{j}_1"),
                        ]
                        for hi, h in enumerate((h0, h1)):
                            pb = 64 * hi
                            for ic in range(2):
                                icol = ib0 + ic * 512
                                nc.tensor.matmul(
                                    Sp[hi][:, ic * 512 : ic * 512 + 512],
                                    kTp[pb : pb + 64, pr, j * 128 : (j + 1) * 128],
                                    qTp[pb : pb + 64, pr, icol : icol + 512],
                                    start=True,
                                    stop=True,
                                )
                        for hi, h in enumerate((h0, h1)):
                            A = ap.tile([128, 1024], F32R, tag="A")
                            nc.scalar.activation(
                                out=A[:], in_=Sp[hi][:], func=EXP_FN, scale=SCALE
                            )
                            for ic in range(2):
                                nc.tensor.matmul(
                                    Op[hi][:, ic * 512 : ic * 512 + 512],
                                    v_aug[:, j, h, :],
                                    A[:, ic * 512 : ic * 512 + 512],
                                    start=(j == 0),
                                    stop=(j == 15),
                                )
                    for hi, h in enumerate((h0, h1)):
                        pb = 64 * hi
                        r = rcp.tile([128, 1024], F32, tag="r")
                        nc.vector.reciprocal(out=r[64:128, :], in_=Op[hi][64:128, :])
                        nc.vector.tensor_tensor(
                            out=attT[pb : pb + 64, pr, ib0 : ib0 + 1024],
                            in0=Op[hi][0:64, :],
                            in1=r[64:128, :],
                            op=mybir.AluOpType.mult,
                        )
                if dbg and iblk == 1:
                    pass
                # out-proj for this i-block
                for it in range(8):
                    irow = ib0 + it * 128
                    yps = sps.tile([128, 1024], F32, tag="s")
                    for uc in range(2):
                        for ot in range(2):
                            nc.tensor.matmul(
                                yps[:, uc * 512 : uc * 512 + 512],
                                attT[:, ot, irow : irow + 128],
                                wo_sb[:, ot, uc * 512 : uc * 512 + 512],
                                start=(ot == 0),
                                stop=(ot == 1),
                            )
                    ty = yp.tile([128, 1024], F32, tag="y")
                    nc.vector.tensor_copy(ty[:], yps[:])
                    nc.sync.dma_start(out=y[irow : irow + 128, :], in_=ty[:])

            if dbg:
                nc.sync.dma_start(out=qTp_d[:], in_=qTp[:].bitcast(F32))
                nc.sync.dma_start(out=kTp_d[:], in_=kTp[:].bitcast(F32))
                nc.sync.dma_start(out=vaug_d[:], in_=v_aug[:].bitcast(F32))
                nc.sync.dma_start(out=attT_d[:], in_=attT[:].bitcast(F32))

    _split_waits(nc)
    nc.finalize()
    return nc


_NC = None
LAST_RESULT = None


def kernel(**inputs) -> np.ndarray:
    global _NC, LAST_RESULT
    if _NC is None:
        _NC = _build()

    x = np.asarray(inputs["x"], dtype=np.float32)
    queries = np.asarray(inputs["queries"], dtype=np.float32)
    wq = np.asarray(inputs["wq"], dtype=np.float32)
    wk = np.asarray(inputs["wk"], dtype=np.float32)
    wv = np.asarray(inputs["wv"], dtype=np.float32)
    wo = np.asarray(inputs["wo"], dtype=np.float32)
    bq = np.asarray(inputs["bq"], dtype=np.float32)
    bk = np.asarray(inputs["bk"], dtype=np.float32)
    bv = np.asarray(inputs["bv"], dtype=np.float32)
    bo = np.asarray(inputs["bo"], dtype=np.float32)

    qT = [np.ascontiguousarray(queries[b].T) for b in range(B)]
    xT = [np.ascontiguousarray(x[b].T) for b in range(B)]
    ones64 = np.ones(64, dtype=np.float32)

    in_maps = []
    for c in range(8):
        b, hg = c // 4, c % 4
        sl = slice(hg * O, (hg + 1) * O)
        in_maps.append(
            {
                "qT": qT[b],
                "xT": xT[b],
                "wqT": np.ascontiguousarray(wq[sl, :].T),
                "wkT": np.ascontiguousarray(wk[sl, :].T),
                "wvT": np.ascontiguousarray(wv[sl, :].T),
                "woT": np.ascontiguousarray(wo[:, sl].T),
                "bqt": np.ascontiguousarray(bq[sl].reshape(2, 128).T),
                "bkt": np.ascontiguousarray(bk[sl].reshape(2, 128).T),
                "bv": np.ascontiguousarray(bv[sl]),
                "ones": ones64,
            }
        )

    res = run_bass_kernel_spmd(_NC, in_maps, core_ids=list(range(8)))
    LAST_RESULT = res

    out = np.empty((B, N, C), dtype=np.float32)
    for b in range(B):
        acc = res.results[4 * b]["y"].astype(np.float32).copy()
        for g in range(1, 4):
            acc += res.results[4 * b + g]["y"]
        out[b] = acc + bo
    return out
